# revision 11
# baseline (speedup 1.0000x reference)
"""MinkowskiInstanceNorm (segment instance-norm over 16 sorted segments) on 8 trn2 cores.

Strategy (sharding hint: shard whole instances across devices):
  - 16 segments, 8 cores -> 2 whole segments per core.
  - Each core's input: its 2 segments, each zero-padded to a fixed C rows so
    the single SPMD NEFF has compile-time-static segment boundaries.
  - Per-core inv_counts [1,2] input carries 1/max(count,1) (pure batch_ids
    metadata, computed on CPU during sharding).
  - Pass 1 (per chunk): stream [128, G*64] fp32 tiles (contiguous DMA),
    square on ScalarE (ACT), segment-sum x and x^2 on the PE via
    ones[128,1].T @ tile matmuls (float32r, full rate for N>=256),
    accumulated into two [1,512] PSUM banks.
  - Stats: reduce PSUM g-partials -> sum/sumsq [1,64]; mean/var/istd;
    A = istd*weight, B = bias - mean*A; broadcast A,B to [128,64].
  - Pass 2 (per chunk): re-stream tiles, out = x*A + B with two DVE
    tensor_tensor ops (A/B broadcast via zero-stride middle dim), DMA out.
  - Unshard on CPU, dropping the padded rows.

Everything is DMA-bound (~200 MB/core over ~358 GB/s => ~560 us target).
"""

import math
import os

import numpy as np

NUM_SEGMENTS = 16
N_CORES = 8
SEGS_PER_CORE = NUM_SEGMENTS // N_CORES  # 2
CH = 64
EPS = 1e-8

# Set by kernel() after each run, for test harness inspection.
last_results = None


def _build_nc(C, G=32):
    """Build the Bass program for one core: 2 chunks of C rows (C % 128 == 0),
    big tiles of G row-blocks ([128, G*CH] fp32)."""
    import concourse.bass as bass
    import concourse.tile as tile
    from concourse import bacc, mybir

    f32 = mybir.dt.float32
    bf16 = mybir.dt.bfloat16
    assert C % 128 == 0
    R = 128 * G  # rows per big tile
    nbig = (C + R - 1) // R
    assert nbig >= 2 or C == R * nbig

    # Bacc (not plain Bass): its compile() legalizes multi-wait instructions
    # (generate_event_semaphores), which walrus requires on TRN2.
    nc = bacc.Bacc("TRN2")
    feats = nc.dram_tensor(
        "feats", [SEGS_PER_CORE * C, CH], f32, kind="ExternalInput"
    ).ap()
    invc = nc.dram_tensor(
        "invc", [1, SEGS_PER_CORE], f32, kind="ExternalInput"
    ).ap()
    weight = nc.dram_tensor("weight", [1, CH], f32, kind="ExternalInput").ap()
    bias = nc.dram_tensor("bias", [1, CH], f32, kind="ExternalInput").ap()
    out = nc.dram_tensor(
        "out", [SEGS_PER_CORE * C, CH], f32, kind="ExternalOutput"
    ).ap()

    with tile.TileContext(nc) as tc:
        with (
            tc.tile_pool(name="xin", bufs=6) as xin_pool,
            tc.tile_pool(name="xsq", bufs=3) as xsq_pool,
            tc.tile_pool(name="small", bufs=1) as small,
            tc.tile_pool(name="stats", bufs=2) as stats,
            tc.tile_pool(name="ab", bufs=2) as ab_pool,
            tc.tile_pool(name="psum", bufs=2, space="PSUM") as psum_pool,
            tc.tile_pool(name="dram", bufs=2, space="DRAM") as dram_pool,
        ):
            # One-time loads / constants
            w_sb = small.tile([1, CH], f32)
            nc.sync.dma_start(out=w_sb[:], in_=weight)
            b_sb = small.tile([1, CH], f32)
            nc.sync.dma_start(out=b_sb[:], in_=bias)
            ic_sb = small.tile([1, SEGS_PER_CORE], f32)
            nc.sync.dma_start(out=ic_sb[:], in_=invc)
            ones_sb = small.tile([128, 1], bf16)
            nc.vector.memset(ones_sb[:], 1.0)
            eps_sb = small.tile([1, 1], f32)
            nc.vector.memset(eps_sb[:], EPS)

            for s in range(SEGS_PER_CORE):
                base = s * C

                # ---------------- Pass 1: segment sums ----------------
                PSW = min(512, G * CH)  # psum accumulator width
                psum_x = psum_pool.tile([1, PSW], f32, tag="px")
                psum_xx = psum_pool.tile([1, PSW], f32, tag="pxx")
                first_x = True
                first_xx = True
                for i in range(nbig):
                    r0 = base + i * R
                    rows = min(R, base + C - r0)
                    g = rows // 128
                    F = g * CH
                    # bf16 cast during the SWDGE load: stats only (pass 2
                    # re-reads full fp32), keeps the PE matmuls at full rate.
                    xt = xin_pool.tile([128, G * CH], bf16, tag="xbf")
                    src = feats[r0 : r0 + rows, :].rearrange(
                        "(p g) c -> p (g c)", p=128
                    )
                    nc.gpsimd.dma_start(out=xt[:, :F], in_=src)
                    sq = xsq_pool.tile([128, G * CH], bf16, tag="sq")
                    nc.scalar.square(sq[:, :F], xt[:, :F])
                    last_tile = i == nbig - 1
                    for j0 in range(0, F, PSW):
                        n = min(PSW, F - j0)
                        last_j = j0 + PSW >= F
                        nc.tensor.matmul(
                            psum_x[0:1, 0:n],
                            ones_sb[:],
                            xt[:, j0 : j0 + n],
                            start=first_x,
                            stop=last_tile and last_j,
                        )
                        first_x = False
                    for j0 in range(0, F, PSW):
                        n = min(PSW, F - j0)
                        last_j = j0 + PSW >= F
                        nc.tensor.matmul(
                            psum_xx[0:1, 0:n],
                            ones_sb[:],
                            sq[:, j0 : j0 + n],
                            start=first_xx,
                            stop=last_tile and last_j,
                        )
                        first_xx = False

                # ---------------- Stats ----------------
                gsub = PSW // CH  # g-partials folded in the psum accumulator
                sum_x = stats.tile([1, CH], f32, tag="sumx")
                nc.vector.tensor_reduce(
                    sum_x[:],
                    psum_x[:].rearrange("p (g c) -> p c g", c=CH),
                    axis=mybir.AxisListType.X,
                    op=mybir.AluOpType.add,
                )
                sum_xx = stats.tile([1, CH], f32, tag="sumxx")
                nc.vector.tensor_reduce(
                    sum_xx[:],
                    psum_xx[:].rearrange("p (g c) -> p c g", c=CH),
                    axis=mybir.AxisListType.X,
                    op=mybir.AluOpType.add,
                )
                ic_view = ic_sb[0:1, s : s + 1].to_broadcast((1, CH))
                mean = stats.tile([1, CH], f32, tag="mean")
                nc.vector.tensor_mul(mean[:], sum_x[:], ic_view)
                msq = stats.tile([1, CH], f32, tag="msq")
                nc.vector.tensor_mul(msq[:], sum_xx[:], ic_view)
                var = stats.tile([1, CH], f32, tag="var")
                nc.vector.tensor_mul(var[:], mean[:], mean[:])
                nc.vector.tensor_sub(var[:], msq[:], var[:])
                sd = stats.tile([1, CH], f32, tag="sd")
                nc.scalar.activation(
                    sd[:],
                    var[:],
                    mybir.ActivationFunctionType.Sqrt,
                    bias=eps_sb[:],
                    scale=1.0,
                )
                istd = stats.tile([1, CH], f32, tag="istd")
                nc.vector.reciprocal(istd[:], sd[:])
                # Pack A = istd*w and B = bias - mean*A into one [1, 2*CH]
                # vector, bounce through DRAM to broadcast across partitions
                # (SBUF-source partition-broadcast DMA is not supported).
                ab_vec = stats.tile([1, 2 * CH], f32, tag="abvec")
                nc.vector.tensor_mul(ab_vec[:, 0:CH], istd[:], w_sb[:])
                nc.vector.tensor_mul(ab_vec[:, CH:], mean[:], ab_vec[:, 0:CH])
                nc.vector.tensor_sub(ab_vec[:, CH:], b_sb[:], ab_vec[:, CH:])
                ab_dram = dram_pool.tile([1, 2 * CH], f32, tag="abdram")
                nc.gpsimd.dma_start(out=ab_dram[:], in_=ab_vec[:])
                ab_bc = ab_pool.tile([128, 2 * CH], f32, tag="abc")
                nc.sync.dma_start(
                    out=ab_bc[:], in_=ab_dram[:].to_broadcast((128, 2 * CH))
                )
                a_bc = ab_bc[:, 0:CH]
                b_bc = ab_bc[:, CH:]

                # ---------------- Pass 2: normalize ----------------
                for i in range(nbig):
                    r0 = base + i * R
                    rows = min(R, base + C - r0)
                    g = rows // 128
                    F = g * CH
                    yt = xin_pool.tile([128, G * CH], f32, tag="x")
                    src = feats[r0 : r0 + rows, :].rearrange(
                        "(p g) c -> p (g c)", p=128
                    )
                    nc.sync.dma_start(out=yt[:, :F], in_=src)
                    yv = yt[:, :F].rearrange("p (g c) -> p g c", c=CH)
                    a_view = bass.AP(
                        tensor=a_bc.tensor,
                        offset=a_bc.offset,
                        ap=[a_bc.ap[0], [0, g], a_bc.ap[1]],
                    )
                    b_view = bass.AP(
                        tensor=b_bc.tensor,
                        offset=b_bc.offset,
                        ap=[b_bc.ap[0], [0, g], b_bc.ap[1]],
                    )
                    nc.vector.tensor_mul(yv, yv, a_view)
                    nc.vector.tensor_add(yv, yv, b_view)
                    dst = out[r0 : r0 + rows, :].rearrange(
                        "(p g) c -> p (g c)", p=128
                    )
                    nc.gpsimd.dma_start(out=dst, in_=yt[:, :F])

    nc.compile()
    return nc


def kernel(feats, batch_ids, weight, bias):
    global last_results
    from concourse.bass_utils import run_bass_kernel_spmd

    feats = np.ascontiguousarray(np.asarray(feats, dtype=np.float32))
    batch_ids = np.asarray(batch_ids, dtype=np.int32)
    weight = np.ascontiguousarray(np.asarray(weight, dtype=np.float32))
    bias = np.ascontiguousarray(np.asarray(bias, dtype=np.float32))

    n = feats.shape[0]
    counts = np.bincount(batch_ids, minlength=NUM_SEGMENTS)
    starts = np.concatenate([[0], np.cumsum(counts)]).astype(np.int64)
    C = max(128, int(math.ceil(counts.max() / 128)) * 128)

    nc = _build_nc(C)

    in_maps = []
    for core in range(N_CORES):
        fp = np.zeros((SEGS_PER_CORE * C, CH), dtype=np.float32)
        icv = np.zeros((1, SEGS_PER_CORE), dtype=np.float32)
        for s in range(SEGS_PER_CORE):
            seg = SEGS_PER_CORE * core + s
            c0, c1 = starts[seg], starts[seg + 1]
            fp[s * C : s * C + (c1 - c0)] = feats[c0:c1]
            icv[0, s] = 1.0 / max(c1 - c0, 1)
        in_maps.append(
            {"feats": fp, "invc": icv, "weight": weight, "bias": bias}
        )

    trace = bool(os.environ.get("BASS_TRACE"))
    last_results = run_bass_kernel_spmd(
        nc, in_maps, core_ids=list(range(N_CORES)), trace=trace
    )

    out = np.empty((n, CH), dtype=np.float32)
    for core in range(N_CORES):
        o = last_results.results[core]["out"]
        for s in range(SEGS_PER_CORE):
            seg = SEGS_PER_CORE * core + s
            c0, c1 = starts[seg], starts[seg + 1]
            out[c0:c1] = o[s * C : s * C + (c1 - c0)]
    return out


# revision 15
# speedup vs baseline: 1.5415x; 1.5415x over previous
"""MinkowskiInstanceNorm (segment instance-norm over 16 sorted segments) on 8 trn2 cores.

Strategy (sharding hint: shard whole instances across devices):
  - 16 segments, 8 cores -> 2 whole segments per core.
  - Each core's input: its 2 segments, each zero-padded to a fixed C rows so
    the single SPMD NEFF has compile-time-static segment boundaries.
  - Per-core inv_counts [1,2] input carries 1/max(count,1) (pure batch_ids
    metadata, computed on CPU during sharding).
  - Pass 1 (per chunk): stream [128, G*64] fp32 tiles (contiguous DMA),
    cast to an fp16 SBUF-resident cache (ScalarE), square on DVE, and
    segment-sum x / x^2 on the PE via ones[128,1].T @ tile matmuls into
    two [1,512] PSUM accumulators.
  - Stats: reduce PSUM g-partials -> sum/sumsq [1,64]; mean/var/istd;
    A = istd*weight, B = bias - mean*A; replicate as fp16 [128, G*64].
  - Pass 2 (per chunk): out = cached_x*A + B, two fp16 DVE tensor_tensor
    ops in place in the cache tile, then SWDGE cast-store fp16->fp32.
    No HBM re-read: per-core traffic is read 67.5 MB + write 67.5 MB.
  - Unshard on CPU, dropping the padded rows.
"""

import math
import os

import numpy as np

NUM_SEGMENTS = 16
N_CORES = 8
SEGS_PER_CORE = NUM_SEGMENTS // N_CORES  # 2
CH = 64
EPS = 1e-8

# Set by kernel() after each run, for test harness inspection.
last_results = None


def _build_nc(C, G=32):
    """Build the Bass program for one core: 2 chunks of C rows (C % 128 == 0),
    big tiles of G row-blocks ([128, G*CH])."""
    import concourse.bass as bass
    import concourse.tile as tile
    from concourse import bacc, mybir

    f32 = mybir.dt.float32
    f16 = mybir.dt.float16
    assert C % 128 == 0
    R = 128 * G  # rows per big tile
    nbig = (C + R - 1) // R
    FB = G * CH  # full big-tile free size

    # Bacc (not plain Bass): its compile() legalizes multi-wait instructions
    # (generate_event_semaphores), which walrus requires on TRN2.
    nc = bacc.Bacc("TRN2")
    feats = nc.dram_tensor(
        "feats", [SEGS_PER_CORE * C, CH], f32, kind="ExternalInput"
    ).ap()
    invc = nc.dram_tensor(
        "invc", [1, SEGS_PER_CORE], f32, kind="ExternalInput"
    ).ap()
    weight = nc.dram_tensor("weight", [1, CH], f32, kind="ExternalInput").ap()
    bias = nc.dram_tensor("bias", [1, CH], f32, kind="ExternalInput").ap()
    out = nc.dram_tensor(
        "out", [SEGS_PER_CORE * C, CH], f32, kind="ExternalOutput"
    ).ap()

    with tile.TileContext(nc) as tc:
        with (
            tc.tile_pool(name="cache", bufs=nbig) as cache_pool,
            tc.tile_pool(name="xin", bufs=3) as xin_pool,
            tc.tile_pool(name="xsq", bufs=2) as xsq_pool,
            tc.tile_pool(name="small", bufs=1) as small,
            tc.tile_pool(name="stats", bufs=2) as stats,
            tc.tile_pool(name="ab", bufs=2) as ab_pool,
            tc.tile_pool(name="psum", bufs=2, space="PSUM") as psum_pool,
            tc.tile_pool(name="dram", bufs=2, space="DRAM") as dram_pool,
        ):
            # One-time loads / constants
            w_sb = small.tile([1, CH], f32)
            nc.sync.dma_start(out=w_sb[:], in_=weight)
            b_sb = small.tile([1, CH], f32)
            nc.sync.dma_start(out=b_sb[:], in_=bias)
            ic_sb = small.tile([1, SEGS_PER_CORE], f32)
            nc.sync.dma_start(out=ic_sb[:], in_=invc)
            ones_sb = small.tile([128, 1], f16)
            nc.vector.memset(ones_sb[:], 1.0)
            eps_sb = small.tile([1, 1], f32)
            nc.vector.memset(eps_sb[:], EPS)

            for s in range(SEGS_PER_CORE):
                base = s * C

                # ---------------- Pass 1: fp16 cache + segment sums ----------
                PSW = min(512, FB)  # psum accumulator width
                psum_x = psum_pool.tile([1, PSW], f32, tag="px")
                psum_xx = psum_pool.tile([1, PSW], f32, tag="pxx")
                first_x = True
                first_xx = True
                cache_tiles = []
                for i in range(nbig):
                    r0 = base + i * R
                    rows = min(R, base + C - r0)
                    g = rows // 128
                    F = g * CH
                    xt = xin_pool.tile([128, FB], f32, tag="x")
                    src = feats[r0 : r0 + rows, :].rearrange(
                        "(p g) c -> p (g c)", p=128
                    )
                    nc.sync.dma_start(out=xt[:, :F], in_=src)
                    ch_t = cache_pool.tile([128, FB], f16, tag="c")
                    cache_tiles.append(ch_t)
                    nc.scalar.copy(ch_t[:, :F], xt[:, :F])
                    sq = xsq_pool.tile([128, FB], f16, tag="sq")
                    nc.vector.tensor_mul(sq[:, :F], ch_t[:, :F], ch_t[:, :F])
                    last_tile = i == nbig - 1
                    for j0 in range(0, F, PSW):
                        n = min(PSW, F - j0)
                        last_j = j0 + PSW >= F
                        nc.tensor.matmul(
                            psum_x[0:1, 0:n],
                            ones_sb[:],
                            ch_t[:, j0 : j0 + n],
                            start=first_x,
                            stop=last_tile and last_j,
                        )
                        first_x = False
                        nc.tensor.matmul(
                            psum_xx[0:1, 0:n],
                            ones_sb[:],
                            sq[:, j0 : j0 + n],
                            start=first_xx,
                            stop=last_tile and last_j,
                        )
                        first_xx = False

                # ---------------- Stats ----------------
                sum_x = stats.tile([1, CH], f32, tag="sumx")
                nc.vector.tensor_reduce(
                    sum_x[:],
                    psum_x[:].rearrange("p (g c) -> p c g", c=CH),
                    axis=mybir.AxisListType.X,
                    op=mybir.AluOpType.add,
                )
                sum_xx = stats.tile([1, CH], f32, tag="sumxx")
                nc.vector.tensor_reduce(
                    sum_xx[:],
                    psum_xx[:].rearrange("p (g c) -> p c g", c=CH),
                    axis=mybir.AxisListType.X,
                    op=mybir.AluOpType.add,
                )
                ic_view = ic_sb[0:1, s : s + 1].to_broadcast((1, CH))
                mean = stats.tile([1, CH], f32, tag="mean")
                nc.vector.tensor_mul(mean[:], sum_x[:], ic_view)
                msq = stats.tile([1, CH], f32, tag="msq")
                nc.vector.tensor_mul(msq[:], sum_xx[:], ic_view)
                var = stats.tile([1, CH], f32, tag="var")
                nc.vector.tensor_mul(var[:], mean[:], mean[:])
                nc.vector.tensor_sub(var[:], msq[:], var[:])
                sd = stats.tile([1, CH], f32, tag="sd")
                nc.scalar.activation(
                    sd[:],
                    var[:],
                    mybir.ActivationFunctionType.Sqrt,
                    bias=eps_sb[:],
                    scale=1.0,
                )
                istd = stats.tile([1, CH], f32, tag="istd")
                nc.vector.reciprocal(istd[:], sd[:])
                # Pack A = istd*w and B = bias - mean*A as fp16, bounce through
                # DRAM, and replicate into [128, G*CH] fp16 operands (step-1
                # layout keeps the pass-2 tensor_tensor in 2x mode).
                ab_vec = stats.tile([1, 2 * CH], f32, tag="abvec")
                nc.vector.tensor_mul(ab_vec[:, 0:CH], istd[:], w_sb[:])
                nc.vector.tensor_mul(ab_vec[:, CH:], mean[:], ab_vec[:, 0:CH])
                nc.vector.tensor_sub(ab_vec[:, CH:], b_sb[:], ab_vec[:, CH:])
                ab_f16 = stats.tile([1, 2 * CH], f16, tag="abf16")
                nc.vector.tensor_copy(ab_f16[:], ab_vec[:])
                ab_dram = dram_pool.tile([1, 2 * CH], f16, tag="abdram")
                nc.gpsimd.dma_start(out=ab_dram[:], in_=ab_f16[:])
                ab_bc = ab_pool.tile([128, 2 * CH], f16, tag="abbc")
                nc.sync.dma_start(
                    out=ab_bc[:], in_=ab_dram[:].to_broadcast((128, 2 * CH))
                )
                # Replicate x G on-chip (DVE zero-stride input) so the pass-2
                # tensor_tensor operands are contiguous step-1 fp16 (2x mode).
                ab_rep = ab_pool.tile([128, 2, G, CH], f16, tag="abrep")
                ab_bc_ap = ab_bc[:]
                for h in range(2):
                    rep_src = bass.AP(
                        tensor=ab_bc_ap.tensor,
                        offset=ab_bc_ap.offset + h * CH,
                        ap=[ab_bc_ap.ap[0], [0, G], [1, CH]],
                    )
                    nc.vector.tensor_copy(ab_rep[:, h, :, :], rep_src)
                a_rep = ab_rep[:, 0, :, :].rearrange("p g c -> p (g c)")
                b_rep = ab_rep[:, 1, :, :].rearrange("p g c -> p (g c)")

                # ---------------- Pass 2: normalize from the fp16 cache ------
                for i in range(nbig):
                    r0 = base + i * R
                    rows = min(R, base + C - r0)
                    g = rows // 128
                    F = g * CH
                    ch_t = cache_tiles[i]
                    nc.vector.tensor_mul(
                        ch_t[:, :F], ch_t[:, :F], a_rep[:, :F]
                    )
                    nc.vector.tensor_add(
                        ch_t[:, :F], ch_t[:, :F], b_rep[:, :F]
                    )
                    dst = out[r0 : r0 + rows, :].rearrange(
                        "(p g) c -> p (g c)", p=128
                    )
                    nc.gpsimd.dma_start(out=dst, in_=ch_t[:, :F])

    nc.compile()
    return nc


def kernel(feats, batch_ids, weight, bias):
    global last_results
    from concourse.bass_utils import run_bass_kernel_spmd

    feats = np.ascontiguousarray(np.asarray(feats, dtype=np.float32))
    batch_ids = np.asarray(batch_ids, dtype=np.int32)
    weight = np.ascontiguousarray(np.asarray(weight, dtype=np.float32))
    bias = np.ascontiguousarray(np.asarray(bias, dtype=np.float32))

    n = feats.shape[0]
    counts = np.bincount(batch_ids, minlength=NUM_SEGMENTS)
    starts = np.concatenate([[0], np.cumsum(counts)]).astype(np.int64)
    C = max(128, int(math.ceil(counts.max() / 128)) * 128)

    nc = _build_nc(C)

    in_maps = []
    for core in range(N_CORES):
        fp = np.zeros((SEGS_PER_CORE * C, CH), dtype=np.float32)
        icv = np.zeros((1, SEGS_PER_CORE), dtype=np.float32)
        for s in range(SEGS_PER_CORE):
            seg = SEGS_PER_CORE * core + s
            c0, c1 = starts[seg], starts[seg + 1]
            fp[s * C : s * C + (c1 - c0)] = feats[c0:c1]
            icv[0, s] = 1.0 / max(c1 - c0, 1)
        in_maps.append(
            {"feats": fp, "invc": icv, "weight": weight, "bias": bias}
        )

    trace = bool(os.environ.get("BASS_TRACE"))
    last_results = run_bass_kernel_spmd(
        nc, in_maps, core_ids=list(range(N_CORES)), trace=trace
    )

    out = np.empty((n, CH), dtype=np.float32)
    for core in range(N_CORES):
        o = last_results.results[core]["out"]
        for s in range(SEGS_PER_CORE):
            seg = SEGS_PER_CORE * core + s
            c0, c1 = starts[seg], starts[seg + 1]
            out[c0:c1] = o[s * C : s * C + (c1 - c0)]
    return out
